# revision 1
# baseline (speedup 1.0000x reference)
"""Trainium2 Bass kernel for nn_BaseEmbedder (retrieval_knn).

For each of 4096 query embeddings: find the 5 nearest of 65536 db embeddings
(Euclidean) and produce the inverse-distance-weighted sum of their auxiliary
features.  SPMD on 8 NeuronCores: queries sharded 512/core, db+aux replicated.

Per core (512 queries = 4 q-tiles of 128 partitions):
  - Scan (bf16): negS[q,j] = q.x_j - 0.5|x_j|^2 via K=34 augmented bf16
    matmuls (rows 32/33 carry the -0.5|x|^2 bias split hi/lo).  Two matmul
    streams run concurrently on PE row-groups 0 and 64 (even/odd 1024-col
    supers).
  - 4-way fold: DVE tensor_tensor(max) folds each group of four 1024-col
    supers (two pair-folds + one merge) into zfold[s] = max of the 4 values,
    16384 folded columns covering the whole db per q-tile.
  - Candidates: one max8 over zfold gives the top-8 folded values; max_index
    recovers their fold slots (needles are in-window, exact f32 match).  Each
    slot maps to FOUR db rows (the fold group); all are gathered, so no
    disambiguation is needed.
  - Exact refinement (f32): a host-prepared table
    row[g*1024+u] = [x,aux of rows (4g+m)*1024+u for m=0..3, then 4x |x|^2]
    is gathered per winning slot via per-partition indirect DMA.  Exact
    distances for all 32 candidates are recomputed on-chip; top-5 by
    threshold (5th-largest of 2*q.x - |x|^2); weights 1/(d+eps) normalized;
    weighted aux sum.

The bf16 scan only nominates candidates; all selection/weight math is exact
f32, so the result matches the f32 reference to ~1e-6.
"""

import numpy as np
import ml_dtypes

from concourse import bass, mybir
from concourse.tile import TileContext
from concourse.bass_utils import run_bass_kernel_spmd

F32 = mybir.dt.float32
BF16 = mybir.dt.bfloat16
U32 = mybir.dt.uint32
I32 = mybir.dt.int32

N_CORES = 8
NQ = 4096
NDB = 65536
D = 32
DAUG = 34   # 32 dims + bias row + bias-residual row (bf16 split)
K = 5
EPS = 1e-6

NQ_CORE = NQ // N_CORES          # 512
CHUNK = 512                      # db columns per matmul (one PSUM bank)
SUPER = 1024                     # db columns per PSUM tile / fold operand
RG_B = 64                        # partition base of the second PE row-group
PV = 260                         # paired row: 4x [x(32) aux(32)] + 4x |x|^2


def build_nc(nq_core=NQ_CORE, ndb=NDB):
    n_qt = nq_core // 128
    n_grp = ndb // (4 * SUPER)           # 4-super fold groups (16)
    fold_w = ndb // 4                    # folded columns (16384, one window)
    assert fold_w <= 16384
    ncand = 4 * 8                        # 8 needles x 4 rows per fold slot

    nc = bass.Bass()
    qT = nc.declare_dram_parameter("qT_aug", [DAUG, nq_core], BF16, isOutput=False)
    qf = nc.declare_dram_parameter("qf", [nq_core, D], F32, isOutput=False)
    qsq = nc.declare_dram_parameter("qsq", [nq_core, 1], F32, isOutput=False)
    dbT = nc.declare_dram_parameter("dbT_aug", [DAUG, ndb], BF16, isOutput=False)
    pairt = nc.declare_dram_parameter("pair_table", [ndb // 4, PV], F32,
                                      isOutput=False)
    out = nc.declare_dram_parameter("out", [nq_core, D], F32, isOutput=True)

    with TileContext(nc) as tc:
        with (
            tc.tile_pool(name="zf", bufs=1) as zfp,
            tc.tile_pool(name="db", bufs=6) as dbp,
            tc.tile_pool(name="sbA", bufs=8) as sap,
            tc.tile_pool(name="sbB", bufs=8) as sbp,
            tc.tile_pool(name="tf", bufs=2) as tfp,
            tc.tile_pool(name="psA", bufs=2, space="PSUM") as pspA,
            tc.tile_pool(name="psB", bufs=2, space="PSUM") as pspB,
            tc.tile_pool(name="sm", bufs=2) as sp,
            tc.tile_pool(name="g", bufs=1) as gp,
        ):
            for t in range(n_qt):
                # queries live on row groups 0 and RG_B so two matmul streams
                # run concurrently on the PE array
                qt = sp.tile([128, 128], BF16, tag="qt")
                nc.sync.dma_start(out=qt[0:DAUG, :],
                                  in_=qT[:, t * 128:(t + 1) * 128])
                nc.sync.dma_start(out=qt[RG_B:RG_B + DAUG, :],
                                  in_=qT[:, t * 128:(t + 1) * 128])
                qs = sp.tile([128, 1], F32, tag="qs")
                nc.sync.dma_start(out=qs[:], in_=qsq[t * 128:(t + 1) * 128, :])
                qft = sp.tile([128, D], F32, tag="qft")
                nc.sync.dma_start(out=qft[:], in_=qf[t * 128:(t + 1) * 128, :])

                candv = sp.tile([128, 8], F32, tag="candv")
                gxa = gp.tile([128, 8, PV], F32, tag="gxa")

                zfold = zfp.tile([128, fold_w], F32)
                for g in range(n_grp):
                    th = []
                    for h in range(2):
                        offa = (4 * g + 2 * h) * SUPER
                        rhs = dbp.tile([128, SUPER], BF16)
                        nc.sync.dma_start(out=rhs[0:DAUG, :],
                                          in_=dbT[:, offa:offa + SUPER])
                        nc.sync.dma_start(
                            out=rhs[RG_B:RG_B + DAUG, :],
                            in_=dbT[:, offa + SUPER:offa + 2 * SUPER])
                        psA = pspA.tile([128, SUPER], F32, tag="psA")
                        psB = pspB.tile([128, SUPER], F32, tag="psB")
                        for m in range(SUPER // CHUNK):
                            sl = slice(m * CHUNK, (m + 1) * CHUNK)
                            nc.tensor.matmul(out=psA[:, sl],
                                             lhsT=qt[0:DAUG, :],
                                             rhs=rhs[0:DAUG, sl],
                                             start=True, stop=True,
                                             tile_position=(0, 0))
                            nc.tensor.matmul(out=psB[:, sl],
                                             lhsT=qt[RG_B:RG_B + DAUG, :],
                                             rhs=rhs[RG_B:RG_B + DAUG, sl],
                                             start=True, stop=True,
                                             tile_position=(RG_B, 0))
                        sbA = sap.tile([128, SUPER], F32)
                        nc.scalar.copy(out=sbA[:], in_=psA[:])
                        sbB = sbp.tile([128, SUPER], F32)
                        nc.scalar.copy(out=sbB[:], in_=psB[:])
                        t1 = tfp.tile([128, SUPER], F32, tag=f"t{h}")
                        nc.vector.tensor_tensor(out=t1[:], in0=sbA[:],
                                                in1=sbB[:],
                                                op=mybir.AluOpType.max)
                        th.append(t1)
                    nc.vector.tensor_tensor(
                        out=zfold[:, g * SUPER:(g + 1) * SUPER],
                        in0=th[0][:], in1=th[1][:], op=mybir.AluOpType.max)
                # top-8 folded values + their fold slots (one window = all db)
                w8 = candv[:, 0:8]
                nc.vector.max(out=w8, in_=zfold[:])
                pos = sp.tile([128, 8], U32, tag="pos")
                nc.vector.max_index(out=pos[:], in_max=w8, in_values=zfold[:])
                ji = sp.tile([128, 8], I32, tag="ji")
                nc.vector.tensor_copy(ji[:], pos[:])
                for i in range(8):
                    nc.gpsimd.indirect_dma_start(
                        out=gxa[:, i, :], out_offset=None, in_=pairt[:],
                        in_offset=bass.IndirectOffsetOnAxis(
                            ap=ji[:, i:i + 1], axis=0))

                # ---- exact f32 refinement over the 32 candidates ----
                # gxa row: 4x [x(32) aux(32)] then 4x |x|^2
                base = gxa[:, :, 0:8 * D].rearrange("p c (h v) -> p c h v", h=4)
                gx = base[:, :, :, 0:D]
                ga = base[:, :, :, D:2 * D]
                xsq = gxa[:, :, 8 * D:8 * D + 4]          # [128, 8, 4]
                # dots[q, c, h] = q . x
                pr = gp.tile([128, 8, 4, D], F32, tag="pr")
                nc.vector.tensor_tensor(
                    out=pr[:], in0=gx,
                    in1=qft[:].unsqueeze(1).unsqueeze(1)
                              .to_broadcast([128, 8, 4, D]),
                    op=mybir.AluOpType.mult)
                dots = sp.tile([128, 8, 4], F32, tag="dots")
                nc.vector.tensor_reduce(out=dots[:], in_=pr[:],
                                        axis=mybir.AxisListType.X,
                                        op=mybir.AluOpType.add)
                # neg2 = 2*dots - xsq  (dsq = qsq - neg2)
                neg2 = sp.tile([128, ncand], F32, tag="neg2")
                nc.vector.scalar_tensor_tensor(
                    out=neg2[:].rearrange("p (c h) -> p c h", h=4),
                    in0=dots[:], scalar=2.0, in1=xsq,
                    op0=mybir.AluOpType.mult, op1=mybir.AluOpType.subtract)
                t8 = sp.tile([128, 8], F32, tag="t8")
                nc.vector.max(out=t8[:], in_=neg2[:])
                mask = sp.tile([128, ncand], F32, tag="mask")
                nc.vector.tensor_scalar(mask[:], neg2[:], t8[:, 4:5], None,
                                        op0=mybir.AluOpType.is_ge)
                dsq = sp.tile([128, ncand], F32, tag="dsq")
                nc.vector.tensor_scalar(dsq[:], neg2[:], -1.0, qs[:, 0:1],
                                        op0=mybir.AluOpType.mult,
                                        op1=mybir.AluOpType.add)
                nc.vector.tensor_scalar_max(dsq[:], dsq[:], 0.0)
                dist = sp.tile([128, ncand], F32, tag="dist")
                nc.scalar.sqrt(out=dist[:], in_=dsq[:])
                nc.vector.tensor_scalar_add(dist[:], dist[:], EPS)
                rec = sp.tile([128, ncand], F32, tag="rec")
                nc.vector.reciprocal(out=rec[:], in_=dist[:])
                wgt = sp.tile([128, ncand], F32, tag="wgt")
                nc.vector.tensor_tensor(out=wgt[:], in0=rec[:], in1=mask[:],
                                        op=mybir.AluOpType.mult)
                wsum = sp.tile([128, 1], F32, tag="wsum")
                nc.vector.tensor_reduce(out=wsum[:], in_=wgt[:],
                                        axis=mybir.AxisListType.X,
                                        op=mybir.AluOpType.add)
                winv = sp.tile([128, 1], F32, tag="winv")
                nc.vector.reciprocal(out=winv[:], in_=wsum[:])

                # weighted sum of gathered aux rows
                prod = gp.tile([128, 8, 4, D], F32, tag="prod")
                nc.vector.tensor_tensor(
                    out=prod[:], in0=ga,
                    in1=wgt[:].rearrange("p (c h) -> p c h", h=4).unsqueeze(-1)
                              .to_broadcast([128, 8, 4, D]),
                    op=mybir.AluOpType.mult)
                acc = sp.tile([128, D], F32, tag="accr")
                nc.vector.tensor_reduce(
                    out=acc[:],
                    in_=prod[:].rearrange("p i h a -> p a (i h)"),
                    axis=mybir.AxisListType.X, op=mybir.AluOpType.add)
                outt = sp.tile([128, D], F32, tag="outt")
                nc.vector.tensor_scalar(outt[:], acc[:], winv[:, 0:1], None,
                                        op0=mybir.AluOpType.mult)
                nc.sync.dma_start(out=out[t * 128:(t + 1) * 128, :], in_=outt[:])

    split_multi_waits(nc)
    return nc


def split_multi_waits(nc):
    """The walrus build in this container supports a single sync-wait per
    instruction; Tile's tail drain carries one wait per live proc.  Split
    any multi-wait instruction into single-wait NoOps ahead of it."""
    for f in nc.m.functions:
        for blk in f.blocks:
            newinsts = []
            for ins in blk.instructions:
                si = ins.sync_info
                if si is not None and si.on_wait and len(si.on_wait) > 1:
                    waits = list(si.on_wait)
                    for k, w in enumerate(waits[:-1]):
                        nop = mybir.InstNoOp(name=f"{ins.name}-ws{k}", ins=[],
                                             outs=[])
                        nop.engine = ins.engine
                        nop.sync_info = mybir.SyncInfo(on_wait=[w], on_update=[])
                        newinsts.append(nop)
                    ins.sync_info = mybir.SyncInfo(on_wait=[waits[-1]],
                                                   on_update=list(si.on_update))
                newinsts.append(ins)
            blk.instructions = newinsts


def make_in_maps(embedding_features, db_embedding, auxiliary_features):
    q = np.ascontiguousarray(np.asarray(embedding_features, dtype=np.float32))
    db = np.ascontiguousarray(np.asarray(db_embedding, dtype=np.float32))
    aux = np.ascontiguousarray(np.asarray(auxiliary_features, dtype=np.float32))
    ndb = db.shape[0]
    nq_core = q.shape[0] // N_CORES
    bf = ml_dtypes.bfloat16
    bias = -0.5 * (db * db).sum(1)                      # exact f32
    b_hi = bias.astype(bf).astype(np.float32)
    b_lo = (bias - b_hi).astype(bf)
    dbT_aug = np.ascontiguousarray(np.concatenate(
        [db.T.astype(bf), b_hi.astype(bf)[None, :], b_lo[None, :]], axis=0,
        dtype=bf))
    # paired table: fold slot s = g*S + u covers db rows (4g+m)*S + u, m=0..3
    idx = np.arange(ndb // 4)
    g_i = idx // SUPER
    u_i = idx % SUPER
    dbsq = (db * db).sum(1)
    pair_table = np.zeros((ndb // 4, PV), np.float32)
    for m in range(4):
        jm = (4 * g_i + m) * SUPER + u_i
        pair_table[:, 2 * m * D:(2 * m + 1) * D] = db[jm]
        pair_table[:, (2 * m + 1) * D:(2 * m + 2) * D] = aux[jm]
        pair_table[:, 8 * D + m] = dbsq[jm]
    pair_table = np.ascontiguousarray(pair_table)
    in_maps = []
    for c in range(N_CORES):
        qs = q[c * nq_core:(c + 1) * nq_core]
        qT_aug = np.ascontiguousarray(np.concatenate(
            [qs.T.astype(bf), np.ones((2, nq_core), bf)], axis=0, dtype=bf))
        qsq = np.ascontiguousarray((qs * qs).sum(1).reshape(nq_core, 1)
                                   ).astype(np.float32)
        in_maps.append({"qT_aug": qT_aug, "qf": qs, "qsq": qsq,
                        "dbT_aug": dbT_aug, "pair_table": pair_table})
    return in_maps


_NC_CACHE = {}


def get_nc(nq_core=NQ_CORE, ndb=NDB):
    key = (nq_core, ndb)
    if key not in _NC_CACHE:
        _NC_CACHE[key] = build_nc(nq_core, ndb)
    return _NC_CACHE[key]


def kernel(embedding_features, db_embedding, auxiliary_features):
    in_maps = make_in_maps(embedding_features, db_embedding, auxiliary_features)
    nc = get_nc()
    res = run_bass_kernel_spmd(nc, in_maps, list(range(N_CORES)))
    return np.concatenate([res.results[c]["out"] for c in range(N_CORES)],
                          axis=0).astype(np.float32)



# revision 10
# speedup vs baseline: 1.0659x; 1.0659x over previous
"""Trainium2 Bass kernel for nn_BaseEmbedder (retrieval_knn).

For each of 4096 query embeddings: find the 5 nearest of 65536 db embeddings
(Euclidean) and produce the inverse-distance-weighted sum of their auxiliary
features.  SPMD on 8 NeuronCores: queries sharded 512/core, db+aux replicated.

Per core (512 queries = 4 q-tiles of 128 partitions), v3 design:
  - db (bf16, augmented with split bias rows carrying -0.5|x|^2) is resident
    in SBUF (loaded once, reused by all 4 q-tiles) as 32 tiles [128,1024]:
    rows 0:34 = fold members 0-3, rows 64:98 = members 4-7.
  - Scan: negS[q,p] = q.x_p - 0.5|x_p|^2 via bf16 matmuls on two PE
    row-groups into f32 PSUM.  Repeated LDWEIGHTS are elided (the stationary
    queries stay resident per row-group for a whole q-tile).
  - Fold-8: db is host-reordered so slot s owns rows {s + 8192 m}.  A group
    = 4 PSUM tiles [128,2048] (member-major) covering 1024 slots x 8
    members.  7 of 8 groups: ACT evacuates PSUM -> bf16 slab, DVE folds the
    slab with 2x-rate bf16 tensor_tensor maxes.  1 mixed group per q-tile:
    DVE tensor_reduce folds two tiles straight off PSUM (load balance).
    zfold[128,8192] bf16.
  - Candidates: max8 + max_index over zfold give the top-8 slots (top-5
    rows' slots always rank <= 5 among slot maxes, so top-8 has 3 slots of
    noise margin; max_index assigns duplicate needle values to distinct
    occurrences).  8 slots x 8 members = 64 candidates.
  - Exact refinement (f32): indirect-gather pair_table rows
    [8x(x,aux), 8x|x|^2]; recompute d^2 = |q|^2 - (2 q.x - |x|^2) in f32,
    top-5 by threshold, weights 1/(d+eps), weighted aux sum.

The bf16 scan only nominates candidate slots; all selection/weight math is
exact f32, so the result matches the f32 reference to ~1e-6.
"""

import numpy as np
import ml_dtypes

from concourse import bass, mybir
from concourse.tile import TileContext
from concourse.bass_utils import run_bass_kernel_spmd

F32 = mybir.dt.float32
BF16 = mybir.dt.bfloat16
U32 = mybir.dt.uint32
I32 = mybir.dt.int32

N_CORES = 8
NQ = 4096
NDB = 65536
D = 32
DAUG = 34   # 32 dims + bias row + bias-residual row (bf16 split)
K = 5
EPS = 1e-6

NQ_CORE = NQ // N_CORES          # 512
NSLOT = 8192                     # fold slots, 8 members each
MEMB = 8
NGRP = 8                         # groups per q-tile
SGRP = NSLOT // NGRP             # 1024 slots per group
PTILE = 2 * SGRP                 # 2048 cols per PSUM tile (2 members)
RG_B = 64                        # partition base of the second PE row-group
PV = MEMB * 2 * D + MEMB         # 520: 8x[x(32) aux(32)] then 8x|x|^2
NCAND = 8 * MEMB                 # 64 candidates

SKIP_REPEAT_LDWEIGHTS = True


def build_nc(nq_core=NQ_CORE, ndb=NDB):
    n_qt = nq_core // 128

    nc = bass.Bass()
    qT = nc.declare_dram_parameter("qT_aug", [DAUG, nq_core], BF16, isOutput=False)
    qf = nc.declare_dram_parameter("qf", [nq_core, D], F32, isOutput=False)
    qsq = nc.declare_dram_parameter("qsq", [nq_core, 1], F32, isOutput=False)
    dbA = nc.declare_dram_parameter("dbA", [DAUG, ndb // 2], BF16, isOutput=False)
    dbB = nc.declare_dram_parameter("dbB", [DAUG, ndb // 2], BF16, isOutput=False)
    pairt = nc.declare_dram_parameter("pair_table", [NSLOT, PV], F32,
                                      isOutput=False)
    out = nc.declare_dram_parameter("out", [nq_core, D], F32, isOutput=True)

    with TileContext(nc) as tc:
        with (
            tc.tile_pool(name="db", bufs=32) as dbp,
            tc.tile_pool(name="zf", bufs=2) as zfp,
            tc.tile_pool(name="slab", bufs=2) as slp,
            tc.tile_pool(name="ft", bufs=1) as ftp,
            tc.tile_pool(name="ps", bufs=2, space="PSUM") as psp,
            tc.tile_pool(name="sm", bufs=2) as sp,
            tc.tile_pool(name="g", bufs=1) as gp,
        ):
            # ---- persistent db load (once, reused by all q-tiles) ----
            db_tiles = []
            for i in range(32):
                dbt = dbp.tile([128, SGRP], BF16)
                nc.sync.dma_start(out=dbt[0:DAUG, :],
                                  in_=dbA[:, i * SGRP:(i + 1) * SGRP])
                nc.scalar.dma_start(out=dbt[RG_B:RG_B + DAUG, :],
                                    in_=dbB[:, i * SGRP:(i + 1) * SGRP])
                db_tiles.append(dbt)

            for t in range(n_qt):
                qt = sp.tile([128, 128], BF16, tag="qt")
                nc.sync.dma_start(out=qt[0:DAUG, :],
                                  in_=qT[:, t * 128:(t + 1) * 128])
                nc.sync.dma_start(out=qt[RG_B:RG_B + DAUG, :],
                                  in_=qT[:, t * 128:(t + 1) * 128])
                qs = sp.tile([128, 1], F32, tag="qs")
                nc.sync.dma_start(out=qs[:], in_=qsq[t * 128:(t + 1) * 128, :])
                qft = sp.tile([128, D], F32, tag="qft")
                nc.sync.dma_start(out=qft[:], in_=qf[t * 128:(t + 1) * 128, :])

                zfold = zfp.tile([128, NSLOT], BF16, tag="zf")
                for g in range(NGRP):
                    mixed = (g == 3)
                    # psum tile k: k=0,1 from row-group A (members 0-3),
                    # k=2,3 from row-group B (members 4-7); tile k covers
                    # db tiles (4g + 2(k%2)) and (4g + 2(k%2) + 1)
                    slab = slp.tile([128, 4 * PTILE], BF16, tag="slab",
                                    name="slab")
                    hs = slab[:, 0:2 * PTILE] if mixed else None
                    za = [None, None]
                    for k in range(4):
                        ps = psp.tile([128, PTILE], F32, tag="ps")
                        lo = k < 2
                        r0 = 0 if lo else RG_B
                        tp = (0, 0) if lo else (RG_B, 0)
                        for h in range(2):
                            dbt = db_tiles[4 * g + 2 * (k % 2) + h]
                            for m in range(2):
                                sl = slice(h * SGRP + m * 512,
                                           h * SGRP + (m + 1) * 512)
                                nc.tensor.matmul(
                                    out=ps[:, sl],
                                    lhsT=qt[r0:r0 + DAUG, :],
                                    rhs=dbt[r0:r0 + DAUG,
                                            m * 512:(m + 1) * 512],
                                    start=True, stop=True, tile_position=tp)
                        if mixed and lo:
                            # fold members (2k, 2k+1) straight off PSUM
                            zz = ftp.tile([128, SGRP], BF16, tag=f"za{k}", name="zz")
                            nc.vector.tensor_reduce(
                                out=zz[:],
                                in_=ps[:].rearrange("p (m s) -> p s m", m=2),
                                axis=mybir.AxisListType.X,
                                op=mybir.AluOpType.max)
                            za[k] = zz
                        elif mixed:
                            nc.scalar.copy(
                                out=hs[:, (k - 2) * PTILE:(k - 1) * PTILE],
                                in_=ps[:])
                        else:
                            nc.scalar.copy(
                                out=slab[:, k * PTILE:(k + 1) * PTILE],
                                in_=ps[:])
                    zslice = zfold[:, g * SGRP:(g + 1) * SGRP]
                    if mixed:
                        zA = ftp.tile([128, SGRP], BF16, tag="zA")
                        nc.vector.tensor_tensor(out=zA[:], in0=za[0][:],
                                                in1=za[1][:],
                                                op=mybir.AluOpType.max)
                        hf = ftp.tile([128, PTILE], BF16, tag="hf")
                        nc.vector.tensor_tensor(
                            out=hf[:], in0=hs[:, 0:PTILE],
                            in1=hs[:, PTILE:2 * PTILE],
                            op=mybir.AluOpType.max)
                        zB = ftp.tile([128, SGRP], BF16, tag="zB")
                        nc.vector.tensor_tensor(out=zB[:], in0=hf[:, 0:SGRP],
                                                in1=hf[:, SGRP:PTILE],
                                                op=mybir.AluOpType.max)
                        nc.vector.tensor_tensor(out=zslice, in0=zA[:],
                                                in1=zB[:],
                                                op=mybir.AluOpType.max)
                    else:
                        f2 = ftp.tile([128, 2 * PTILE], BF16, tag="f2")
                        nc.vector.tensor_tensor(
                            out=f2[:], in0=slab[:, 0:2 * PTILE],
                            in1=slab[:, 2 * PTILE:4 * PTILE],
                            op=mybir.AluOpType.max)
                        f4 = ftp.tile([128, PTILE], BF16, tag="f4")
                        nc.vector.tensor_tensor(
                            out=f4[:], in0=f2[:, 0:PTILE],
                            in1=f2[:, PTILE:2 * PTILE],
                            op=mybir.AluOpType.max)
                        nc.vector.tensor_tensor(out=zslice, in0=f4[:, 0:SGRP],
                                                in1=f4[:, SGRP:PTILE],
                                                op=mybir.AluOpType.max)

                # ---- top-8 fold slots ----
                w8 = sp.tile([128, 8], BF16, tag="w8")
                nc.vector.max(out=w8[:], in_=zfold[:])
                pos = sp.tile([128, 8], U32, tag="pos")
                nc.vector.max_index(out=pos[:], in_max=w8[:], in_values=zfold[:])
                ji = sp.tile([128, 8], I32, tag="ji")
                nc.vector.tensor_copy(ji[:], pos[:])
                gxa = gp.tile([128, 8, PV], F32, tag="gxa")
                for i in range(8):
                    nc.gpsimd.indirect_dma_start(
                        out=gxa[:, i, :], out_offset=None, in_=pairt[:],
                        in_offset=bass.IndirectOffsetOnAxis(
                            ap=ji[:, i:i + 1], axis=0))

                # ---- exact f32 refinement over the 64 candidates ----
                base = gxa[:, :, 0:2 * D * MEMB].rearrange(
                    "p c (h v) -> p c h v", h=MEMB)
                gx = base[:, :, :, 0:D]
                ga = base[:, :, :, D:2 * D]
                xsq = gxa[:, :, 2 * D * MEMB:2 * D * MEMB + MEMB]  # [128,8,8]
                pr = gp.tile([128, 8, MEMB, D], F32, tag="pr")
                nc.vector.tensor_tensor(
                    out=pr[:], in0=gx,
                    in1=qft[:].unsqueeze(1).unsqueeze(1)
                              .to_broadcast([128, 8, MEMB, D]),
                    op=mybir.AluOpType.mult)
                dots = sp.tile([128, 8, MEMB], F32, tag="dots")
                nc.vector.tensor_reduce(out=dots[:], in_=pr[:],
                                        axis=mybir.AxisListType.X,
                                        op=mybir.AluOpType.add)
                neg2 = sp.tile([128, NCAND], F32, tag="neg2")
                nc.vector.scalar_tensor_tensor(
                    out=neg2[:].rearrange("p (c h) -> p c h", h=MEMB),
                    in0=dots[:], scalar=2.0, in1=xsq,
                    op0=mybir.AluOpType.mult, op1=mybir.AluOpType.subtract)
                t8 = sp.tile([128, 8], F32, tag="t8")
                nc.vector.max(out=t8[:], in_=neg2[:])
                mask = sp.tile([128, NCAND], F32, tag="mask")
                nc.vector.tensor_scalar(mask[:], neg2[:], t8[:, 4:5], None,
                                        op0=mybir.AluOpType.is_ge)
                dsq = sp.tile([128, NCAND], F32, tag="dsq")
                nc.vector.tensor_scalar(dsq[:], neg2[:], -1.0, qs[:, 0:1],
                                        op0=mybir.AluOpType.mult,
                                        op1=mybir.AluOpType.add)
                nc.vector.tensor_scalar_max(dsq[:], dsq[:], 0.0)
                dist = sp.tile([128, NCAND], F32, tag="dist")
                nc.scalar.sqrt(out=dist[:], in_=dsq[:])
                nc.vector.tensor_scalar_add(dist[:], dist[:], EPS)
                rec = sp.tile([128, NCAND], F32, tag="rec")
                nc.vector.reciprocal(out=rec[:], in_=dist[:])
                wgt = sp.tile([128, NCAND], F32, tag="wgt")
                nc.vector.tensor_tensor(out=wgt[:], in0=rec[:], in1=mask[:],
                                        op=mybir.AluOpType.mult)
                wsum = sp.tile([128, 1], F32, tag="wsum")
                nc.vector.tensor_reduce(out=wsum[:], in_=wgt[:],
                                        axis=mybir.AxisListType.X,
                                        op=mybir.AluOpType.add)
                winv = sp.tile([128, 1], F32, tag="winv")
                nc.vector.reciprocal(out=winv[:], in_=wsum[:])

                prod = gp.tile([128, 8, MEMB, D], F32, tag="prod")
                nc.vector.tensor_tensor(
                    out=prod[:], in0=ga,
                    in1=wgt[:].rearrange("p (c h) -> p c h", h=MEMB)
                              .unsqueeze(-1).to_broadcast([128, 8, MEMB, D]),
                    op=mybir.AluOpType.mult)
                acc = sp.tile([128, D], F32, tag="accr")
                nc.vector.tensor_reduce(
                    out=acc[:],
                    in_=prod[:].rearrange("p i h a -> p a (i h)"),
                    axis=mybir.AxisListType.X, op=mybir.AluOpType.add)
                outt = sp.tile([128, D], F32, tag="outt")
                nc.vector.tensor_scalar(outt[:], acc[:], winv[:, 0:1], None,
                                        op0=mybir.AluOpType.mult)
                nc.sync.dma_start(out=out[t * 128:(t + 1) * 128, :], in_=outt[:])

    split_multi_waits(nc)
    if SKIP_REPEAT_LDWEIGHTS:
        skip_repeat_ldweights(nc)
    return nc


def skip_repeat_ldweights(nc):
    """Consecutive matmuls on the same PE row-group with identical stationary
    weights don't need to reload them; mark ldweights=False on the repeats."""
    for f in nc.m.functions:
        for blk in f.blocks:
            last = {}
            for ins in blk.instructions:
                if isinstance(ins, mybir.InstMatmult):
                    tp = tuple(ins.tile_position or (0, 0))
                    key = repr(ins.ins[1])
                    if last.get(tp) == key:
                        ins.ldweights = False
                    else:
                        last[tp] = key


def split_multi_waits(nc):
    """The walrus build in this container supports a single sync-wait per
    instruction; Tile's tail drain carries one wait per live proc.  Split
    any multi-wait instruction into single-wait NoOps ahead of it."""
    for f in nc.m.functions:
        for blk in f.blocks:
            newinsts = []
            for ins in blk.instructions:
                si = ins.sync_info
                if si is not None and si.on_wait and len(si.on_wait) > 1:
                    waits = list(si.on_wait)
                    for k, w in enumerate(waits[:-1]):
                        nop = mybir.InstNoOp(name=f"{ins.name}-ws{k}", ins=[],
                                             outs=[])
                        nop.engine = ins.engine
                        nop.sync_info = mybir.SyncInfo(on_wait=[w], on_update=[])
                        newinsts.append(nop)
                    ins.sync_info = mybir.SyncInfo(on_wait=[waits[-1]],
                                                   on_update=list(si.on_update))
                newinsts.append(ins)
            blk.instructions = newinsts


def make_in_maps(embedding_features, db_embedding, auxiliary_features):
    q = np.ascontiguousarray(np.asarray(embedding_features, dtype=np.float32))
    db = np.ascontiguousarray(np.asarray(db_embedding, dtype=np.float32))
    aux = np.ascontiguousarray(np.asarray(auxiliary_features, dtype=np.float32))
    ndb = db.shape[0]
    nq_core = q.shape[0] // N_CORES
    bf = ml_dtypes.bfloat16
    bias = -0.5 * (db * db).sum(1)                      # exact f32
    b_hi = bias.astype(bf).astype(np.float32)
    b_lo = (bias - b_hi).astype(bf)
    dbT_aug = np.ascontiguousarray(np.concatenate(
        [db.T.astype(bf), b_hi.astype(bf)[None, :], b_lo[None, :]], axis=0,
        dtype=bf))                                      # [34, ndb]

    # slot s owns rows {s + NSLOT*m}.  Stream order (per half): group g
    # (1024 slots), member-major: col j of group g = m_local*SGRP + s_local,
    # row = (SGRP*g + s_local) + NSLOT*m   (A half: m=0..3, B half: m=4..7)
    s_loc = np.arange(SGRP)
    cols = []
    for half in range(2):
        rows = np.empty((NGRP, 4, SGRP), np.int64)
        for g in range(NGRP):
            for mm in range(4):
                m = half * 4 + mm
                rows[g, mm] = (SGRP * g + s_loc) + NSLOT * m
        cols.append(rows.reshape(-1))
    dbA_s = np.ascontiguousarray(dbT_aug[:, cols[0]])
    dbB_s = np.ascontiguousarray(dbT_aug[:, cols[1]])

    dbsq = (db * db).sum(1)
    s_all = np.arange(NSLOT)
    pair_table = np.zeros((NSLOT, PV), np.float32)
    for m in range(MEMB):
        jm = s_all + NSLOT * m
        pair_table[:, 2 * m * D:(2 * m + 1) * D] = db[jm]
        pair_table[:, (2 * m + 1) * D:(2 * m + 2) * D] = aux[jm]
        pair_table[:, 2 * MEMB * D + m] = dbsq[jm]
    pair_table = np.ascontiguousarray(pair_table)

    in_maps = []
    for c in range(N_CORES):
        qs = q[c * nq_core:(c + 1) * nq_core]
        qT_aug = np.ascontiguousarray(np.concatenate(
            [qs.T.astype(bf), np.ones((2, nq_core), bf)], axis=0, dtype=bf))
        qsq = np.ascontiguousarray((qs * qs).sum(1).reshape(nq_core, 1)
                                   ).astype(np.float32)
        in_maps.append({"qT_aug": qT_aug, "qf": qs, "qsq": qsq,
                        "dbA": dbA_s, "dbB": dbB_s, "pair_table": pair_table})
    return in_maps


_NC_CACHE = {}


def get_nc(nq_core=NQ_CORE, ndb=NDB):
    key = (nq_core, ndb)
    if key not in _NC_CACHE:
        _NC_CACHE[key] = build_nc(nq_core, ndb)
    return _NC_CACHE[key]


def kernel(embedding_features, db_embedding, auxiliary_features):
    in_maps = make_in_maps(embedding_features, db_embedding, auxiliary_features)
    nc = get_nc()
    res = run_bass_kernel_spmd(nc, in_maps, list(range(N_CORES)))
    return np.concatenate([res.results[c]["out"] for c in range(N_CORES)],
                          axis=0).astype(np.float32)
